# revision 21
# baseline (speedup 1.0000x reference)
"""MoE ConditionalFeedForward (SwiGLU top-2 of 8 experts) on 8 Trainium2 cores.

Strategy: expert-parallel. Core c owns expert c's weights. The host routes
tokens: all (token, slot) assignments are bucketed by expert; each core runs
the dense SwiGLU FFN for up to C=512 of its expert's tokens (one full-width
matmul block). The handful of assignments beyond 512 per expert ("spill",
~1% of work) is computed on the host. Only activated pairs are computed
(~4x fewer FLOPs than the dense reference).

All matmul data is fp16 (PSUM accumulation is fp32): same 1 cycle/row PE
rate as fp32r but half the HBM traffic, and the compiler enables fast
weight load (FWL) for 16-bit weights so LDWEIGHTS fully hides under the
previous matmul. Layouts are feature-major ("transposed") end to end so the
contraction dim always sits on SBUF partitions and no on-device transposes
are needed:
  phase 1: h1T/h3T[i, t] = sum_d w1T[d, i] * xT[d, t]   (lhsT=w1 chunk, rhs=x)
  fuse:    hT = silu(h1T) * h3T
  phase 2: outT[d, t]    = sum_i w2T[i, d] * hT[i, t]

Phase-2 accumulation alternates between two PSUM banks (kic parity, merged by
a DVE add) so back-to-back matmuls never chain on one bank.

DMA schedule (one HWDGE queue sustains only ~150 GB/s, and phase 1 consumes
~145 GB/s of weights, so the weight stream is split across two free-running
sequencers): w1 rides the sync queue and w3 the gpsimd queue (~72 GB/s
each); x is split by k-chunk parity across the scalar and vector queues so
the whole 2MB lands within the first i-chunk's consumption; w2 rides the
scalar queue with pushes interleaved into phase 1 after the pipeline-fill
window (and pool-gated pushes for the tail d-chunks); out rides the sync
queue, idle once w1 is done. Dummy matmuls on a zeroed tile warm the PE
p-state during the ~8us runtime prologue. ps1/ps2 PSUM pools coexist on
disjoint banks so the phase boundary has no write-after-read wait.
"""

import numpy as np

T, A = 2048, 2
E, I, D = 8, 4096, 2048
N_CORES = 8
KC = D // 128   # 16 contraction chunks of 128 over D
IC = I // 128   # 32 i-chunks of 128
DC = D // 128   # 16 output d-chunks of 128
N_WARM = 6      # PE p-state warmup matmuls (fill the runtime prologue)
W2_BUFS = 7     # w2 d-chunks buffered in SBUF
W2_IC0 = 10     # first phase-1 i-chunk after which w2 pushes interleave

TRACE = False          # set by test harness to capture an NTFF profile
LAST_EXEC_NS = None    # filled when TRACE is set
_CACHE = {}            # compiled program cache keyed by (C, blocks)


def _split_blocks(C):
    """Split C tokens into even-sized matmul free-dim blocks (<=512)."""
    nb = max(1, -(-C // 512))
    base = 2 * (-(-C // (2 * nb)))
    blocks = []
    rem = C
    for _ in range(nb - 1):
        blocks.append(base)
        rem -= base
    blocks.append(rem)
    assert all(b > 0 and b % 2 == 0 for b in blocks) and sum(blocks) == C
    return blocks


def _build_program(C, blocks):
    import concourse.bass as bass
    import concourse.tile as tile
    from concourse import bacc, mybir

    f32 = mybir.dt.float32
    f16 = mybir.dt.float16

    nc = bacc.Bacc("TRN2", target_bir_lowering=False, debug=False,
                   num_devices=N_CORES)
    x_ap = nc.dram_tensor("x", [KC, 128, C], f16, kind="ExternalInput").ap()
    w1_ap = nc.dram_tensor("w1", [IC, 128, KC * 128], f16, kind="ExternalInput").ap()
    w3_ap = nc.dram_tensor("w3", [IC, 128, KC * 128], f16, kind="ExternalInput").ap()
    w2_ap = nc.dram_tensor("w2", [DC, 128, IC * 128], f16, kind="ExternalInput").ap()
    o_ap = nc.dram_tensor("o", [D, C], f32, kind="ExternalOutput").ap()

    boff = np.cumsum([0] + blocks)[:-1]

    with tile.TileContext(nc) as tc:
        with tc.tile_pool(name="xpool", bufs=1) as xpool, \
             tc.tile_pool(name="hpool", bufs=1) as hpool, \
             tc.tile_pool(name="w13", bufs=5) as w13pool, \
             tc.tile_pool(name="w2p", bufs=W2_BUFS) as w2pool, \
             tc.tile_pool(name="w13h", bufs=1) as w13hpool, \
             tc.tile_pool(name="act", bufs=2) as actpool, \
             tc.tile_pool(name="outp", bufs=2) as outpool, \
             tc.tile_pool(name="ps1", bufs=2, space="PSUM") as ps1, \
             tc.tile_pool(name="ps2", bufs=2, space="PSUM") as ps2:

            # Fill-window schedule. All 8 cores hit HBM at once during fill,
            # so arrival order of the first ~4MB is the critical path. The
            # first NHEAD i-chunks of w1/w3 are loaded in kc-quarter tiles
            # (128KB dependency granularity — matmuls start on partial
            # arrivals and stalls stay small, avoiding p-state re-ramp):
            #  - sync queue:   w1 quarters for ic 0..NHEAD-1
            #  - scalar queue: x evens then the last odds
            #  - gpsimd queue: w3 quarters interleaved with early x odds
            NHEAD = 4
            xts = [xpool.tile([128, C], f16, name=f"xt_{kc}") for kc in range(KC)]
            wq1, wq3 = {}, {}
            for ic in range(NHEAD):
                for j in range(4):
                    wq1[(ic, j)] = w13hpool.tile(
                        [128, 4 * 128], f16, tag=f"tw1s{ic}_{j}",
                        name=f"tw1q_{ic}_{j}")
                    wq3[(ic, j)] = w13hpool.tile(
                        [128, 4 * 128], f16, tag=f"tw3s{ic}_{j}",
                        name=f"tw3q_{ic}_{j}")

            def _push_w1q(ic, j):
                nc.sync.dma_start(wq1[(ic, j)][:],
                                  w1_ap[ic, :, j * 512:(j + 1) * 512])

            def _push_w3q(ic, j):
                nc.gpsimd.dma_start(wq3[(ic, j)][:],
                                    w3_ap[ic, :, j * 512:(j + 1) * 512])

            # sync queue: w1 quarters (lead matmul operand first), x13/x15
            # slotted where sync has slack
            for ic, j in [(0, 0), (0, 1), (0, 2), (0, 3), (1, 0), (1, 1),
                          (1, 2), (1, 3)]:
                _push_w1q(ic, j)
            nc.sync.dma_start(xts[13][:], x_ap[13])
            nc.sync.dma_start(xts[15][:], x_ap[15])
            for ic, j in [(2, 0), (2, 1), (2, 2), (2, 3), (3, 0), (3, 1),
                          (3, 2), (3, 3)]:
                _push_w1q(ic, j)
            # scalar queue: x chunks ordered by consumption time
            for kc in (0, 2, 4, 6, 8, 9, 10, 11, 12, 14):
                nc.scalar.dma_start(xts[kc][:], x_ap[kc])
            # gpsimd queue: w3 quarters with the earliest-needed x odds mixed in
            _push_w3q(0, 0)
            nc.gpsimd.dma_start(xts[1][:], x_ap[1])
            nc.gpsimd.dma_start(xts[3][:], x_ap[3])
            _push_w3q(0, 1)
            nc.gpsimd.dma_start(xts[5][:], x_ap[5])
            _push_w3q(0, 2)
            nc.gpsimd.dma_start(xts[7][:], x_ap[7])
            _push_w3q(0, 3)
            for ic in range(1, NHEAD):
                for j in range(4):
                    _push_w3q(ic, j)
            ht = hpool.tile([128, IC * C], f16, name="ht")

            # PE p-state warmup: dummy matmuls on a zeroed tile run during the
            # runtime prologue + first weight-tile DMA, so the clock is ramped
            # when real work arrives
            warm = xpool.tile([128, 512], f16, name="warm")
            nc.vector.memset(warm[:], 0.0)
            pw = [ps2.tile([128, 512], f32, tag=f"po_0_{par}", name=f"pw_{par}")
                  for par in (0, 1)]
            for wi in range(N_WARM):
                nc.tensor.matmul(pw[wi % 2][:], warm[:, :128], warm[:],
                                 start=True, stop=True)

            # w2 tile allocation + push helper: dc -> (tiles, pushed flag)
            w2_tiles = {}

            def w2_push(dc):
                tw2a = w2pool.tile([128, (IC // 2) * 128], f16, tag="tw2a",
                                   name=f"tw2a_{dc}")
                tw2b = w2pool.tile([128, (IC // 2) * 128], f16, tag="tw2b",
                                   name=f"tw2b_{dc}")
                nc.scalar.dma_start(tw2a[:], w2_ap[dc, :, :(IC // 2) * 128])
                nc.scalar.dma_start(tw2b[:], w2_ap[dc, :, (IC // 2) * 128:])
                w2_tiles[dc] = (tw2a, tw2b)

            # ---- phase 1: hT = silu(w1T.T @ x) * (w3T.T @ x), per i-chunk ----
            for ic in range(IC):
                if ic < NHEAD:
                    tw1 = tw3 = None
                else:
                    tw1 = w13pool.tile([128, KC * 128], f16, tag="tw1",
                                       name=f"tw1_{ic}")
                    tw3 = w13pool.tile([128, KC * 128], f16, tag="tw3",
                                       name=f"tw3_{ic}")
                    nc.sync.dma_start(tw1[:], w1_ap[ic])
                    nc.gpsimd.dma_start(tw3[:], w3_ap[ic])
                for g0 in range(0, len(blocks), 2):
                    grp = list(enumerate(blocks))[g0:g0 + 2]
                    p1 = [ps1.tile([128, bn], f32, tag=f"p1_{bi - g0}",
                                   name=f"p1_{ic}_{bi}")
                          for bi, bn in grp]
                    p3 = [ps1.tile([128, bn], f32, tag=f"p3_{bi - g0}",
                                   name=f"p3_{ic}_{bi}")
                          for bi, bn in grp]
                    for kc in range(KC):
                        if ic < NHEAD:
                            wsl1 = wq1[(ic, kc // 4)][:, (kc % 4) * 128:(kc % 4 + 1) * 128]
                            wsl3 = wq3[(ic, kc // 4)][:, (kc % 4) * 128:(kc % 4 + 1) * 128]
                        else:
                            wsl1 = tw1[:, kc * 128:(kc + 1) * 128]
                            wsl3 = tw3[:, kc * 128:(kc + 1) * 128]
                        st, sp = (kc == 0), (kc == KC - 1)
                        for gi, (bi, bn) in enumerate(grp):
                            xsl = xts[kc][:, boff[bi]: boff[bi] + bn]
                            nc.tensor.matmul(p1[gi][:], wsl1, xsl, start=st, stop=sp)
                        for gi, (bi, bn) in enumerate(grp):
                            xsl = xts[kc][:, boff[bi]: boff[bi] + bn]
                            nc.tensor.matmul(p3[gi][:], wsl3, xsl, start=st, stop=sp)
                    for gi, (bi, bn) in enumerate(grp):
                        s1 = actpool.tile([128, bn], f16, tag=f"s1_{bi - g0}",
                                          name=f"s1_{ic}_{bi}")
                        nc.scalar.activation(s1[:], p1[gi][:],
                                             mybir.ActivationFunctionType.Silu)
                        hsl = ht[:, ic * C + boff[bi]: ic * C + boff[bi] + bn]
                        nc.vector.tensor_mul(hsl, s1[:], p3[gi][:])
                # interleave w2 pushes on the scalar seq after the fill
                # window: dc 0..W2_BUFS-1 stream during the phase-1 tail
                if ic >= W2_IC0 and (ic - W2_IC0) % 2 == 0:
                    dc = (ic - W2_IC0) // 2
                    if dc < W2_BUFS:
                        w2_push(dc)

            # ---- phase 2: outT = w2T.T @ hT, per d-chunk ----
            for dc in range(DC):
                if dc not in w2_tiles:
                    # tail d-chunks: pool-gated pushes (the buf-free wait
                    # releases as earlier d-chunks retire)
                    w2_push(dc)
                tw2a, tw2b = w2_tiles[dc]
                ot = outpool.tile([128, C], f32, tag="ot", name=f"ot_{dc}")
                for g0 in range(0, len(blocks), 2):
                    grp = list(enumerate(blocks))[g0:g0 + 2]
                    po = {}
                    for gi, (bi, bn) in enumerate(grp):
                        for par in (0, 1):
                            po[(gi, par)] = ps2.tile(
                                [128, bn], f32, tag=f"po_{bi - g0}_{par}",
                                name=f"po_{dc}_{bi}_{par}")
                    for kic in range(IC):
                        half = tw2a if kic < IC // 2 else tw2b
                        j = kic % (IC // 2)
                        wsl = half[:, j * 128:(j + 1) * 128]
                        par = kic % 2
                        st, sp = (kic < 2), (kic >= IC - 2)
                        for gi, (bi, bn) in enumerate(grp):
                            hsl = ht[:, kic * C + boff[bi]: kic * C + boff[bi] + bn]
                            nc.tensor.matmul(po[(gi, par)][:], wsl, hsl,
                                             start=st, stop=sp)
                    # drain: on the last d-chunk, split the psum merge + out
                    # DMA into column halves so the final store pipelines
                    last = dc == DC - 1
                    nsp = 2 if last and min(bn for _, bn in grp) % 4 == 0 else 1
                    for gi, (bi, bn) in enumerate(grp):
                        for sp_i in range(nsp):
                            r0, r1 = sp_i * (bn // nsp), (sp_i + 1) * (bn // nsp)
                            osl = ot[:, boff[bi] + r0:boff[bi] + r1]
                            nc.vector.tensor_copy(osl, po[(gi, 0)][:, r0:r1])
                            nc.vector.tensor_add(osl, osl, po[(gi, 1)][:, r0:r1])
                            if last:
                                nc.sync.dma_start(
                                    o_ap[dc * 128:(dc + 1) * 128,
                                         boff[bi] + r0:boff[bi] + r1], osl)
                if dc < DC - 1:
                    nc.sync.dma_start(o_ap[dc * 128:(dc + 1) * 128, :], ot[:])

    nc.compile()
    return nc


def _run_spmd(nc, in_maps):
    global LAST_EXEC_NS
    from concourse import bass_utils
    if TRACE:
        import sys, types
        try:
            from antenv.axon_hooks import get_axon_ntff_profile_hook  # noqa
        except ImportError:
            from trn_agent_boot.trn_boot import _ntff_profile_via_ctypes
            _hook = _ntff_profile_via_ctypes('/opt/axon/libaxon_pjrt.so')
            m = types.ModuleType("antenv.axon_hooks")
            m.get_axon_ntff_profile_hook = lambda: _hook
            sys.modules["antenv.axon_hooks"] = m
        bass_utils.upload_artifacts = lambda tmpdir: "local://" + tmpdir
    res = bass_utils.run_bass_kernel_spmd(
        nc, in_maps, core_ids=list(range(N_CORES)), trace=TRACE)
    if TRACE:
        LAST_EXEC_NS = res.exec_time_ns
    return res.results


def kernel(x, expert_indices, w1, w2, w3):
    x = np.asarray(x)
    ei = np.asarray(expert_indices)
    w1 = np.asarray(w1)
    w2 = np.asarray(w2)
    w3 = np.asarray(w3)

    # ---- host routing ----
    flat = ei.reshape(-1).astype(np.int64)          # assignment -> expert
    order = np.argsort(flat, kind="stable")         # assignments grouped by expert
    counts = np.bincount(flat, minlength=E)
    off = np.concatenate([[0], np.cumsum(counts)])
    C = int(counts.max())
    C += C % 2                                      # even free dims
    C = max(min(C, 512), 2)                         # cap: spill goes to host
    blocks = tuple(_split_blocks(C))

    key = (C, blocks)
    if key not in _CACHE:
        _CACHE[key] = _build_program(C, list(blocks))
    nc = _CACHE[key]

    # token row lists per expert (first C assignments), padded with token 0;
    # assignments beyond C ("spill", a handful of tokens) are computed on host
    tok = np.zeros((E, C), dtype=np.int64)
    ndev = np.minimum(counts, C)
    for e in range(E):
        rows = order[off[e]:off[e] + ndev[e]] // A
        tok[e, :ndev[e]] = rows

    in_maps = []
    for e in range(E):
        xg = x[tok[e]]                                    # [C, D]
        xT = np.ascontiguousarray(xg.T.astype(np.float16)).reshape(KC, 128, C)
        # w1/w3 [I, D] -> [ic, j, kc, p] -> [ic, p, kc, j]
        w1p = np.ascontiguousarray(
            w1[e].astype(np.float16).reshape(IC, 128, KC, 128).transpose(0, 3, 2, 1)
        ).reshape(IC, 128, KC * 128)
        w3p = np.ascontiguousarray(
            w3[e].astype(np.float16).reshape(IC, 128, KC, 128).transpose(0, 3, 2, 1)
        ).reshape(IC, 128, KC * 128)
        # w2 [D, I] -> [dc, j, kic, p] -> [dc, p, kic, j]
        w2p = np.ascontiguousarray(
            w2[e].astype(np.float16).reshape(DC, 128, IC, 128).transpose(0, 3, 2, 1)
        ).reshape(DC, 128, IC * 128)
        in_maps.append({"x": xT, "w1": w1p, "w3": w3p, "w2": w2p})

    results = _run_spmd(nc, in_maps)

    # ---- host scatter + spill compute ----
    out_flat = np.empty((T * A, D), dtype=np.float32)
    for e in range(E):
        oT = results[e]["o"]                              # [D, C]
        o_e = oT.T                                        # [C, D]
        idx = order[off[e]:off[e] + ndev[e]]
        out_flat[idx] = o_e[:ndev[e]]
        if counts[e] > ndev[e]:
            sidx = order[off[e] + ndev[e]:off[e + 1]]
            xs = x[sidx // A]                             # [s, D]
            h1 = xs @ w1[e].T
            h3 = xs @ w3[e].T
            h = (h1 / (1.0 + np.exp(-h1))) * h3
            out_flat[sidx] = h @ w2[e].T
    return out_flat.reshape(T, A, D)


# revision 26
# speedup vs baseline: 1.0177x; 1.0177x over previous
"""MoE ConditionalFeedForward (SwiGLU top-2 of 8 experts) on 8 Trainium2 cores.

Strategy: expert-parallel. Core c owns expert c's weights. The host routes
tokens: all (token, slot) assignments are bucketed by expert; each core runs
the dense SwiGLU FFN for up to C=512 of its expert's tokens (one full-width
matmul block). The handful of assignments beyond 512 per expert ("spill",
~1% of work) is computed on the host. Only activated pairs are computed
(~4x fewer FLOPs than the dense reference).

All matmul data is fp16 (PSUM accumulation is fp32): same 1 cycle/row PE
rate as fp32r but half the HBM traffic, and the compiler enables fast
weight load (FWL) for 16-bit weights so LDWEIGHTS fully hides under the
previous matmul. Layouts are feature-major ("transposed") end to end so the
contraction dim always sits on SBUF partitions and no on-device transposes
are needed:
  phase 1: h1T/h3T[i, t] = sum_d w1T[d, i] * xT[d, t]   (lhsT=w1 chunk, rhs=x)
  fuse:    hT = silu(h1T) * h3T
  phase 2: outT[d, t]    = sum_i w2T[i, d] * hT[i, t]

Phase-2 accumulation alternates between two PSUM banks (kic parity, merged by
a DVE add) so back-to-back matmuls never chain on one bank.

DMA schedule (one HWDGE queue sustains only ~150 GB/s, and phase 1 consumes
~145 GB/s of weights, so the weight stream is split across two free-running
sequencers): w1 rides the sync queue and w3 the gpsimd queue (~72 GB/s
each); x is split by k-chunk parity across the scalar and vector queues so
the whole 2MB lands within the first i-chunk's consumption; w2 rides the
scalar queue with pushes interleaved into phase 1 after the pipeline-fill
window (and pool-gated pushes for the tail d-chunks); out rides the sync
queue, idle once w1 is done. Dummy matmuls on a zeroed tile warm the PE
p-state during the ~8us runtime prologue. ps1/ps2 PSUM pools coexist on
disjoint banks so the phase boundary has no write-after-read wait.
"""

import numpy as np

T, A = 2048, 2
E, I, D = 8, 4096, 2048
N_CORES = 8
KC = D // 128   # 16 contraction chunks of 128 over D
IC = I // 128   # 32 i-chunks of 128
DC = D // 128   # 16 output d-chunks of 128
N_WARM = 4      # PE p-state warmup matmuls (fill the runtime prologue)
W2_BUFS = 7     # w2 d-chunks buffered in SBUF
W2_IC0 = 10     # first phase-1 i-chunk after which w2 pushes interleave
NHEAD = 3       # leading i-chunks loaded as kc-quarter tiles

TRACE = False          # set by test harness to capture an NTFF profile
LAST_EXEC_NS = None    # filled when TRACE is set
_CACHE = {}            # compiled program cache keyed by (C, blocks)


def _split_blocks(C):
    """Split C tokens into even-sized matmul free-dim blocks (<=512)."""
    nb = max(1, -(-C // 512))
    base = 2 * (-(-C // (2 * nb)))
    blocks = []
    rem = C
    for _ in range(nb - 1):
        blocks.append(base)
        rem -= base
    blocks.append(rem)
    assert all(b > 0 and b % 2 == 0 for b in blocks) and sum(blocks) == C
    return blocks


def _build_program(C, blocks):
    import concourse.bass as bass
    import concourse.tile as tile
    from concourse import bacc, mybir

    f32 = mybir.dt.float32
    f16 = mybir.dt.float16

    nc = bacc.Bacc("TRN2", target_bir_lowering=False, debug=False,
                   num_devices=N_CORES)
    x_ap = nc.dram_tensor("x", [KC, 128, C], f16, kind="ExternalInput").ap()
    w1_ap = nc.dram_tensor("w1", [IC, 128, KC * 128], f16, kind="ExternalInput").ap()
    w3_ap = nc.dram_tensor("w3", [IC, 128, KC * 128], f16, kind="ExternalInput").ap()
    w2_ap = nc.dram_tensor("w2", [DC, 128, IC * 128], f16, kind="ExternalInput").ap()
    o_ap = nc.dram_tensor("o", [D, C], f32, kind="ExternalOutput").ap()

    boff = np.cumsum([0] + blocks)[:-1]

    with tile.TileContext(nc) as tc:
        with tc.tile_pool(name="xpool", bufs=1) as xpool, \
             tc.tile_pool(name="hpool", bufs=1) as hpool, \
             tc.tile_pool(name="w13", bufs=5) as w13pool, \
             tc.tile_pool(name="w2p", bufs=W2_BUFS) as w2pool, \
             tc.tile_pool(name="w13h", bufs=1) as w13hpool, \
             tc.tile_pool(name="act", bufs=2) as actpool, \
             tc.tile_pool(name="outp", bufs=2) as outpool, \
             tc.tile_pool(name="ps1", bufs=2, space="PSUM") as ps1, \
             tc.tile_pool(name="ps2", bufs=2, space="PSUM") as ps2:

            # Fill-window schedule. All 8 cores hit HBM at once during fill,
            # so arrival order of the first ~4MB is the critical path. The
            # first NHEAD i-chunks of w1/w3 are loaded in kc-quarter tiles
            # (128KB dependency granularity — matmuls start on partial
            # arrivals and stalls stay small, avoiding p-state re-ramp):
            #  - sync queue:   w1 quarters for ic 0..NHEAD-1
            #  - scalar queue: x evens then the last odds
            #  - gpsimd queue: w3 quarters interleaved with early x odds
            xts = [xpool.tile([128, C], f16, name=f"xt_{kc}") for kc in range(KC)]
            wq1, wq3 = {}, {}
            for ic in range(NHEAD):
                for j in range(4):
                    wq1[(ic, j)] = w13hpool.tile(
                        [128, 4 * 128], f16, tag=f"tw1s{ic}_{j}",
                        name=f"tw1q_{ic}_{j}")
                    wq3[(ic, j)] = w13hpool.tile(
                        [128, 4 * 128], f16, tag=f"tw3s{ic}_{j}",
                        name=f"tw3q_{ic}_{j}")

            def _push_w1q(ic, j):
                nc.sync.dma_start(wq1[(ic, j)][:],
                                  w1_ap[ic, :, j * 512:(j + 1) * 512])

            def _push_w3q(ic, j):
                nc.gpsimd.dma_start(wq3[(ic, j)][:],
                                    w3_ap[ic, :, j * 512:(j + 1) * 512])

            # sync queue: w1 quarters (lead matmul operand first), x13/x15
            # slotted where sync has slack
            wq_all = [(ic, j) for ic in range(NHEAD) for j in range(4)]
            for ic, j in wq_all[:8]:
                _push_w1q(ic, j)
            nc.sync.dma_start(xts[13][:], x_ap[13])
            nc.sync.dma_start(xts[15][:], x_ap[15])
            for ic, j in wq_all[8:]:
                _push_w1q(ic, j)
            # scalar queue: x chunks ordered by consumption time
            for kc in (0, 2, 4, 6, 8, 9, 10, 11, 12, 14):
                nc.scalar.dma_start(xts[kc][:], x_ap[kc])
            # gpsimd queue: w3 quarters with the earliest-needed x odds mixed in
            _push_w3q(0, 0)
            nc.gpsimd.dma_start(xts[1][:], x_ap[1])
            nc.gpsimd.dma_start(xts[3][:], x_ap[3])
            _push_w3q(0, 1)
            nc.gpsimd.dma_start(xts[5][:], x_ap[5])
            _push_w3q(0, 2)
            nc.gpsimd.dma_start(xts[7][:], x_ap[7])
            _push_w3q(0, 3)
            for ic, j in wq_all[4:]:
                _push_w3q(ic, j)
            ht = hpool.tile([128, IC * C], f16, name="ht")

            # PE p-state warmup: dummy matmuls on a zeroed tile run during the
            # runtime prologue + first weight-tile DMA, so the clock is ramped
            # when real work arrives
            warm = xpool.tile([128, 512], f16, name="warm")
            nc.vector.memset(warm[:], 0.0)
            pw = [ps2.tile([128, 512], f32, tag=f"po_0_{par}", name=f"pw_{par}")
                  for par in (0, 1)]
            for wi in range(N_WARM):
                nc.tensor.matmul(pw[wi % 2][:], warm[:, :128], warm[:],
                                 start=True, stop=True)

            # w2 tile allocation + push helper: dc -> (tiles, pushed flag)
            w2_tiles = {}

            def w2_push(dc):
                tw2a = w2pool.tile([128, (IC // 2) * 128], f16, tag="tw2a",
                                   name=f"tw2a_{dc}")
                tw2b = w2pool.tile([128, (IC // 2) * 128], f16, tag="tw2b",
                                   name=f"tw2b_{dc}")
                nc.scalar.dma_start(tw2a[:], w2_ap[dc, :, :(IC // 2) * 128])
                nc.scalar.dma_start(tw2b[:], w2_ap[dc, :, (IC // 2) * 128:])
                w2_tiles[dc] = (tw2a, tw2b)

            # ---- phase 1: hT = silu(w1T.T @ x) * (w3T.T @ x), per i-chunk ----
            for ic in range(IC):
                if ic < NHEAD:
                    tw1 = tw3 = None
                else:
                    tw1 = w13pool.tile([128, KC * 128], f16, tag="tw1",
                                       name=f"tw1_{ic}")
                    tw3 = w13pool.tile([128, KC * 128], f16, tag="tw3",
                                       name=f"tw3_{ic}")
                    nc.sync.dma_start(tw1[:], w1_ap[ic])
                    nc.gpsimd.dma_start(tw3[:], w3_ap[ic])
                for g0 in range(0, len(blocks), 2):
                    grp = list(enumerate(blocks))[g0:g0 + 2]
                    p1 = [ps1.tile([128, bn], f32, tag=f"p1_{bi - g0}",
                                   name=f"p1_{ic}_{bi}")
                          for bi, bn in grp]
                    p3 = [ps1.tile([128, bn], f32, tag=f"p3_{bi - g0}",
                                   name=f"p3_{ic}_{bi}")
                          for bi, bn in grp]
                    for kc in range(KC):
                        if ic < NHEAD:
                            wsl1 = wq1[(ic, kc // 4)][:, (kc % 4) * 128:(kc % 4 + 1) * 128]
                            wsl3 = wq3[(ic, kc // 4)][:, (kc % 4) * 128:(kc % 4 + 1) * 128]
                        else:
                            wsl1 = tw1[:, kc * 128:(kc + 1) * 128]
                            wsl3 = tw3[:, kc * 128:(kc + 1) * 128]
                        st, sp = (kc == 0), (kc == KC - 1)
                        for gi, (bi, bn) in enumerate(grp):
                            xsl = xts[kc][:, boff[bi]: boff[bi] + bn]
                            nc.tensor.matmul(p1[gi][:], wsl1, xsl, start=st, stop=sp)
                        for gi, (bi, bn) in enumerate(grp):
                            xsl = xts[kc][:, boff[bi]: boff[bi] + bn]
                            nc.tensor.matmul(p3[gi][:], wsl3, xsl, start=st, stop=sp)
                    for gi, (bi, bn) in enumerate(grp):
                        s1 = actpool.tile([128, bn], f16, tag=f"s1_{bi - g0}",
                                          name=f"s1_{ic}_{bi}")
                        nc.scalar.activation(s1[:], p1[gi][:],
                                             mybir.ActivationFunctionType.Silu)
                        hsl = ht[:, ic * C + boff[bi]: ic * C + boff[bi] + bn]
                        nc.vector.tensor_mul(hsl, s1[:], p3[gi][:])
                # interleave w2 pushes on the scalar seq after the fill
                # window: dc 0..W2_BUFS-1 stream during the phase-1 tail
                if ic >= W2_IC0 and (ic - W2_IC0) % 2 == 0:
                    dc = (ic - W2_IC0) // 2
                    if dc < W2_BUFS:
                        w2_push(dc)

            # ---- phase 2: outT = w2T.T @ hT, per d-chunk ----
            for dc in range(DC):
                if dc not in w2_tiles:
                    # tail d-chunks: pool-gated pushes (the buf-free wait
                    # releases as earlier d-chunks retire)
                    w2_push(dc)
                tw2a, tw2b = w2_tiles[dc]
                ot = outpool.tile([128, C], f32, tag="ot", name=f"ot_{dc}")
                for g0 in range(0, len(blocks), 2):
                    grp = list(enumerate(blocks))[g0:g0 + 2]
                    po = {}
                    for gi, (bi, bn) in enumerate(grp):
                        for par in (0, 1):
                            po[(gi, par)] = ps2.tile(
                                [128, bn], f32, tag=f"po_{bi - g0}_{par}",
                                name=f"po_{dc}_{bi}_{par}")
                    for kic in range(IC):
                        half = tw2a if kic < IC // 2 else tw2b
                        j = kic % (IC // 2)
                        wsl = half[:, j * 128:(j + 1) * 128]
                        par = kic % 2
                        st, sp = (kic < 2), (kic >= IC - 2)
                        for gi, (bi, bn) in enumerate(grp):
                            hsl = ht[:, kic * C + boff[bi]: kic * C + boff[bi] + bn]
                            nc.tensor.matmul(po[(gi, par)][:], wsl, hsl,
                                             start=st, stop=sp)
                    # drain: on the last d-chunk, split the psum merge + out
                    # DMA into column halves so the final store pipelines
                    last = dc == DC - 1
                    nsp = 2 if last and min(bn for _, bn in grp) % 4 == 0 else 1
                    for gi, (bi, bn) in enumerate(grp):
                        for sp_i in range(nsp):
                            r0, r1 = sp_i * (bn // nsp), (sp_i + 1) * (bn // nsp)
                            osl = ot[:, boff[bi] + r0:boff[bi] + r1]
                            nc.vector.tensor_copy(osl, po[(gi, 0)][:, r0:r1])
                            nc.vector.tensor_add(osl, osl, po[(gi, 1)][:, r0:r1])
                            if last:
                                nc.sync.dma_start(
                                    o_ap[dc * 128:(dc + 1) * 128,
                                         boff[bi] + r0:boff[bi] + r1], osl)
                if dc < DC - 1:
                    nc.sync.dma_start(o_ap[dc * 128:(dc + 1) * 128, :], ot[:])

    nc.compile()
    return nc


def _run_spmd(nc, in_maps):
    global LAST_EXEC_NS
    from concourse import bass_utils
    if TRACE:
        import sys, types
        try:
            from antenv.axon_hooks import get_axon_ntff_profile_hook  # noqa
        except ImportError:
            from trn_agent_boot.trn_boot import _ntff_profile_via_ctypes
            _hook = _ntff_profile_via_ctypes('/opt/axon/libaxon_pjrt.so')
            m = types.ModuleType("antenv.axon_hooks")
            m.get_axon_ntff_profile_hook = lambda: _hook
            sys.modules["antenv.axon_hooks"] = m
        bass_utils.upload_artifacts = lambda tmpdir: "local://" + tmpdir
    res = bass_utils.run_bass_kernel_spmd(
        nc, in_maps, core_ids=list(range(N_CORES)), trace=TRACE)
    if TRACE:
        LAST_EXEC_NS = res.exec_time_ns
    return res.results


def kernel(x, expert_indices, w1, w2, w3):
    x = np.asarray(x)
    ei = np.asarray(expert_indices)
    w1 = np.asarray(w1)
    w2 = np.asarray(w2)
    w3 = np.asarray(w3)

    # ---- host routing ----
    flat = ei.reshape(-1).astype(np.int64)          # assignment -> expert
    order = np.argsort(flat, kind="stable")         # assignments grouped by expert
    counts = np.bincount(flat, minlength=E)
    off = np.concatenate([[0], np.cumsum(counts)])
    C = int(counts.max())
    C += C % 2                                      # even free dims
    C = max(min(C, 512), 2)                         # cap: spill goes to host
    blocks = tuple(_split_blocks(C))

    key = (C, blocks, N_WARM, W2_BUFS, W2_IC0, NHEAD)
    if key not in _CACHE:
        _CACHE[key] = _build_program(C, list(blocks))
    nc = _CACHE[key]

    # token row lists per expert (first C assignments), padded with token 0;
    # assignments beyond C ("spill", a handful of tokens) are computed on host
    tok = np.zeros((E, C), dtype=np.int64)
    ndev = np.minimum(counts, C)
    for e in range(E):
        rows = order[off[e]:off[e] + ndev[e]] // A
        tok[e, :ndev[e]] = rows

    in_maps = []
    for e in range(E):
        xg = x[tok[e]]                                    # [C, D]
        xT = np.ascontiguousarray(xg.T.astype(np.float16)).reshape(KC, 128, C)
        # w1/w3 [I, D] -> [ic, j, kc, p] -> [ic, p, kc, j]
        w1p = np.ascontiguousarray(
            w1[e].astype(np.float16).reshape(IC, 128, KC, 128).transpose(0, 3, 2, 1)
        ).reshape(IC, 128, KC * 128)
        w3p = np.ascontiguousarray(
            w3[e].astype(np.float16).reshape(IC, 128, KC, 128).transpose(0, 3, 2, 1)
        ).reshape(IC, 128, KC * 128)
        # w2 [D, I] -> [dc, j, kic, p] -> [dc, p, kic, j]
        w2p = np.ascontiguousarray(
            w2[e].astype(np.float16).reshape(DC, 128, IC, 128).transpose(0, 3, 2, 1)
        ).reshape(DC, 128, IC * 128)
        in_maps.append({"x": xT, "w1": w1p, "w3": w3p, "w2": w2p})

    results = _run_spmd(nc, in_maps)

    # ---- host scatter + spill compute ----
    out_flat = np.empty((T * A, D), dtype=np.float32)
    for e in range(E):
        oT = results[e]["o"]                              # [D, C]
        o_e = oT.T                                        # [C, D]
        idx = order[off[e]:off[e] + ndev[e]]
        out_flat[idx] = o_e[:ndev[e]]
        if counts[e] > ndev[e]:
            sidx = order[off[e] + ndev[e]:off[e + 1]]
            xs = x[sidx // A]                             # [s, D]
            h1 = xs @ w1[e].T
            h3 = xs @ w3[e].T
            h = (h1 / (1.0 + np.exp(-h1))) * h3
            out_flat[sidx] = h @ w2[e].T
    return out_flat.reshape(T, A, D)
